# revision 14
# baseline (speedup 1.0000x reference)
"""Trainium2 Bass kernel for a 3x3 stride-1 pad-1 Conv2d.

Problem: x (16, 64, 112, 112) f32, weights (128, 64, 9) f32
         -> out (16, 128, 112, 112) f32  (no bias)

Strategy (8 NeuronCores, data parallel over batch):
  - Each core gets 2 images. Image 0 lives in SBUF partitions 0-63
    (64 input channels), image 1 in partitions 64-127, both stored as a
    zero-padded (114, 114) plane per channel. The zero padding is
    materialized on the host (xp input), so every input DMA is a fully
    contiguous fat-descriptor transfer straight into the padded plane.
  - All off-chip data is fp16 (x, weights, output staging); accumulation
    is fp32 in PSUM. Measured end-to-end rel err ~3.4e-4. fp16 halves
    HBM traffic (10 MB/core vs 20) and enables the PE's fast-weight-load
    path (disabled for fp32), which otherwise bounds the matmul pair
    rate at ~215ns instead of ~191ns.
  - Conv = 9 shift-and-matmul taps accumulated in PSUM: for each tap
    (dy, dx), matmul with lhsT = w[tap] (64 x 128: in-ch x out-ch) and
    rhs = shifted x window (64 x 448: in-ch x 4 output rows).
  - The two images' matmuls use disjoint PE row groups (rows 0-63 vs
    64-127 via tile_position), so they execute concurrently -> together
    they fill the whole 128x128 array despite the 64-deep contraction.
    Measured steady pair cadence 191ns vs the 187ns streaming floor.
  - PE warmup: a single accumulation group of 34 full-128-row dummy
    matmuls over memset scratch bridges body entry to first-data so the
    HAM clock gate (PE at 1.2 GHz until ~3.4us of sustained FULL-ARRAY
    activity; 64-row streams don't count) is open when real work starts.
  - Input bands are single 128-partition DMAs (both images at once).
    Weights are split: taps 0-2 ahead of band 0 on the Sync HWDGE queue
    (gating block 0), taps 3-8 on the Scalar queue in parallel. Later
    bands are serially completion-chained -- the PE consumes rows far
    slower than the chain delivers, and the chain keeps head-window HBM
    bandwidth for the transfers that gate the first matmul.
  - PSUM -> SBUF casts (f32 -> fp16) split per image across DVE and ACT
    so the two copies run concurrently; output bands are stored per 16
    rows with large descriptors (the last band stores per 4-row block on
    both HWDGE queues to shrink the drain tail). The host upcasts the
    fp16 output back to f32.
"""

import numpy as np

import concourse.bass as bass
import concourse.bacc as bacc
import concourse.mybir as mybir
import concourse.tile as tile
from concourse.bass_utils import run_bass_kernel_spmd
from concourse.tile_rust import add_dep_helper

N_CORES = 8
B, C, H, W = 16, 64, 112, 112
O = 128
BPC = B // N_CORES          # images per core
HP = H + 2                  # padded rows per image plane
WP = W + 2                  # padded cols
NTAPS = 9
RPB = 4                     # output rows per block (free dim = 4*112 = 448)
NBLOCKS = H // RPB          # 28
BAND = 16                   # output rows per output band
NBANDS = H // BAND          # 7

F32 = mybir.dt.float32
F16 = mybir.dt.float16

# input bands over padded rows: (first padded row, nrows). The head band
# covers exactly block 0 so the first matmul can start early; later bands
# are completion-chained serially (the PE consumes rows far slower than
# the serial chain delivers them, and the chain keeps the head window's
# HBM bandwidth for the weight + band-0 transfers).
_IN_BANDS = [(0, 6), (6, 12), (18, 16), (34, 16), (50, 16), (66, 16),
             (82, 16), (98, 16)]


def _conv_body(tc, out_ap, xp_ap, w_ap):
    nc = tc.nc
    from contextlib import ExitStack

    with ExitStack() as ctx:
        xpool = ctx.enter_context(tc.tile_pool(name="xb", bufs=1))
        wpool = ctx.enter_context(tc.tile_pool(name="wt", bufs=1))
        pspool = ctx.enter_context(tc.tile_pool(name="ps", bufs=3, space="PSUM"))
        pdpool = ctx.enter_context(tc.tile_pool(name="pd", bufs=1, space="PSUM"))
        opool = ctx.enter_context(tc.tile_pool(name="ob", bufs=4))
        scpool = ctx.enter_context(tc.tile_pool(name="sc", bufs=1))

        # x planes: partitions [64*im, 64*im+64) hold image im, padded.
        xb = xpool.tile([128, HP, WP], F16)
        # weights: wt[p, t, m] = w[m, p % 64, t] (taps replicated per half)
        wt = wpool.tile([128, NTAPS, O], F16)

        # PE warmup: the HAM clock gate only opens after a full 4096-cycle
        # window (3.4us @ 1.2GHz, free-running phase) of uninterrupted PE
        # activity; cold matmuls run at half clock. Dummy matmuls over
        # memset scratch bridge the gap from body entry (~7.7us) until the
        # first real matmul's data is ready (~10.9us): the PE must stay
        # busy THROUGH that point (an idle gap restarts the window), so
        # slightly overshooting and delaying the real stream by <=107ns
        # per surplus dummy beats undershooting, which costs the whole
        # ~1.9us cold-phase penalty.
        # The dummies must be ONE accumulation group: isolated start/stop
        # matmuls pay a ~128-cycle drain between groups, and that ~50% PE
        # duty cycle never trips the HAM busy detector (observed: 30
        # isolated dummies left the gate closed). Memsets go on GpSimd,
        # which is idle and enters the body earliest.
        sw = scpool.tile([128, O], F16)
        sx = scpool.tile([128, O], F16)
        pdum = pdpool.tile([128, RPB * W], F32)
        nc.gpsimd.memset(sw[:], 0.0)
        nc.gpsimd.memset(sx[:], 0.0)
        # Full 128-row dummies: 64-row (half-array) dummy streams were
        # observed NOT to trip the HAM busy detector at all.
        NDUM = 28
        for i in range(NDUM):
            nc.tensor.matmul(pdum[:, 0:O], sw[:], sx[:],
                             start=(i == 0), stop=(i == NDUM - 1),
                             tile_position=(0, 0))

        # Head ordering: a DMA's completion sem lags its last data byte by
        # ~+0.4us when it is FIRST on its HWDGE ring but ~+1.3us when it
        # sits behind another transfer (straggler compounding down the
        # FIFO). So the two transfers that gate the first matmul each go
        # FIRST on their own ring: band 0 on Sync, taps 0-2 on Scalar.
        # The remaining taps split behind them, each landing just before
        # the tap stream needs it.
        nc.scalar.dma_start(out=wt[:, 0:3, :], in_=w_ap[:, 0:3, :])
        nc.scalar.dma_start(out=wt[:, 6:NTAPS, :], in_=w_ap[:, 6:NTAPS, :])

        band_dmas = []
        for bi, (r0, n) in enumerate(_IN_BANDS):
            d = nc.sync.dma_start(
                out=xb[:, r0:r0 + n, :],
                in_=xp_ap[:, r0:r0 + n, :],
            )
            if bi == 0:
                nc.sync.dma_start(out=wt[:, 3:6, :], in_=w_ap[:, 3:6, :])
            if bi >= 2:
                add_dep_helper(d.ins, band_dmas[bi - 1].ins, reason="band chain")
            band_dmas.append(d)

        ob_tiles = {}
        for p in range(NBLOCKS):
            r = RPB * p
            band = r // BAND
            boff = r - band * BAND
            if boff == 0:
                for im in range(BPC):
                    ob_tiles[im] = opool.tile(
                        [128, BAND, W], F16, name=f"ob{im}_{band}", tag=f"ob{im}"
                    )
            ps = [
                pspool.tile([128, RPB, W], F32, tag=f"ps{im}", name=f"ps{im}_{p}")
                for im in range(BPC)
            ]
            for t in range(NTAPS):
                i, j = divmod(t, 3)
                first, last = t == 0, t == NTAPS - 1
                for im in range(BPC):
                    p0 = 64 * im
                    nc.tensor.matmul(
                        ps[im][:],
                        wt[p0:p0 + 64, t, :],
                        xb[p0:p0 + 64, r + i:r + i + RPB, j:j + W],
                        start=first,
                        stop=last,
                        tile_position=(p0, 0),
                    )
            # PSUM -> SBUF casts split across engines: image 0 on DVE,
            # image 1 on ACT (both ~1 elem/cycle from PSUM), so the two
            # copies run concurrently and the end-of-kernel drain chain
            # is one copy deep, not two.
            nc.vector.tensor_copy(ob_tiles[0][:, boff:boff + RPB, :], ps[0][:])
            nc.scalar.copy(ob_tiles[1][:, boff:boff + RPB, :], ps[1][:])
            last_band = band == NBANDS - 1
            if last_band:
                # store per block; image 0 on the Sync queue (input bands
                # have long drained), image 1 on Scalar -> the two final
                # stores run on independent HWDGE rings.
                nc.sync.dma_start(
                    out=out_ap[0, :, r:r + RPB, :],
                    in_=ob_tiles[0][:, boff:boff + RPB, :],
                )
                nc.scalar.dma_start(
                    out=out_ap[1, :, r:r + RPB, :],
                    in_=ob_tiles[1][:, boff:boff + RPB, :],
                )
            elif boff + RPB == BAND:
                for im in range(BPC):
                    nc.scalar.dma_start(
                        out=out_ap[im, :, band * BAND:(band + 1) * BAND, :],
                        in_=ob_tiles[im][:],
                    )


def build_program():
    nc = bacc.Bacc("TRN2", target_bir_lowering=False, num_devices=N_CORES)
    x_t = nc.dram_tensor("xp", [BPC * C, HP, WP], F16, kind="ExternalInput")
    w_t = nc.dram_tensor("wT", [128, NTAPS, O], F16, kind="ExternalInput")
    o_t = nc.dram_tensor("out", [BPC, O, H, W], F16, kind="ExternalOutput")
    with tile.TileContext(nc) as tc:
        _conv_body(tc, o_t.ap(), x_t.ap(), w_t.ap())
    nc.compile()
    return nc


def pack_weights(weights: np.ndarray) -> np.ndarray:
    # (O, C, 9) -> (128, 9, O) with wT[p, t, m] = weights[m, p % 64, t]
    wT = np.ascontiguousarray(np.transpose(weights, (1, 2, 0)))  # (C, 9, O)
    return np.ascontiguousarray(
        np.concatenate([wT, wT], axis=0).astype(np.float16)
    )


def pad_input(x: np.ndarray) -> np.ndarray:
    # (B, C, H, W) -> (B, C, H+2, W+2) zero-padded, fp16
    xp = np.zeros((x.shape[0], x.shape[1], HP, WP), np.float16)
    xp[:, :, 1:1 + H, 1:1 + W] = x
    return xp


def run(x: np.ndarray, weights: np.ndarray, **spmd_kwargs):
    x = np.ascontiguousarray(x, dtype=np.float32)
    w = np.ascontiguousarray(weights, dtype=np.float32)
    wT = pack_weights(w)
    xp = pad_input(x)
    nc = build_program()
    in_maps = [
        {"xp": xp[BPC * i:BPC * (i + 1)].reshape(BPC * C, HP, WP), "wT": wT}
        for i in range(N_CORES)
    ]
    res = run_bass_kernel_spmd(nc, in_maps, list(range(N_CORES)), **spmd_kwargs)
    outs = [
        np.asarray(res.results[i]["out"])
        .reshape(BPC, O, H, W)
        .astype(np.float32)
        for i in range(N_CORES)
    ]
    return np.concatenate(outs, axis=0), res


def kernel(x: np.ndarray, weights: np.ndarray) -> np.ndarray:
    out, _ = run(x, weights)
    return out


# revision 15
# speedup vs baseline: 1.0133x; 1.0133x over previous
"""Trainium2 Bass kernel for a 3x3 stride-1 pad-1 Conv2d.

Problem: x (16, 64, 112, 112) f32, weights (128, 64, 9) f32
         -> out (16, 128, 112, 112) f32  (no bias)

Strategy (8 NeuronCores, data parallel over batch):
  - Each core gets 2 images. Image 0 lives in SBUF partitions 0-63
    (64 input channels), image 1 in partitions 64-127, both stored as a
    zero-padded (114, 114) plane per channel. The zero padding is
    materialized on the host (xp input), so every input DMA is a fully
    contiguous fat-descriptor transfer straight into the padded plane.
  - All off-chip data is fp16 (x, weights, output staging); accumulation
    is fp32 in PSUM. Measured end-to-end rel err ~3.4e-4. fp16 halves
    HBM traffic (10 MB/core vs 20) and enables the PE's fast-weight-load
    path (disabled for fp32), which otherwise bounds the matmul pair
    rate at ~215ns instead of ~191ns.
  - Conv = 9 shift-and-matmul taps accumulated in PSUM: for each tap
    (dy, dx), matmul with lhsT = w[tap] (64 x 128: in-ch x out-ch) and
    rhs = shifted x window (64 x 448: in-ch x 4 output rows).
  - The two images' matmuls use disjoint PE row groups (rows 0-63 vs
    64-127 via tile_position), so they execute concurrently -> together
    they fill the whole 128x128 array despite the 64-deep contraction.
    Measured steady pair cadence 191ns vs the 187ns streaming floor.
  - PE warmup: a single accumulation group of 34 full-128-row dummy
    matmuls over memset scratch bridges body entry to first-data so the
    HAM clock gate (PE at 1.2 GHz until ~3.4us of sustained FULL-ARRAY
    activity; 64-row streams don't count) is open when real work starts.
  - Input bands are single 128-partition DMAs (both images at once).
    Weights are split: taps 0-2 ahead of band 0 on the Sync HWDGE queue
    (gating block 0), taps 3-8 on the Scalar queue in parallel. Later
    bands are serially completion-chained -- the PE consumes rows far
    slower than the chain delivers, and the chain keeps head-window HBM
    bandwidth for the transfers that gate the first matmul.
  - PSUM -> SBUF casts (f32 -> fp16) split per image across DVE and ACT
    so the two copies run concurrently; output bands are stored per 16
    rows with large descriptors (the last band stores per 4-row block on
    both HWDGE queues to shrink the drain tail). The host upcasts the
    fp16 output back to f32.
"""

import numpy as np

import concourse.bass as bass
import concourse.bacc as bacc
import concourse.mybir as mybir
import concourse.tile as tile
from concourse.bass_utils import run_bass_kernel_spmd
from concourse.tile_rust import add_dep_helper

N_CORES = 8
B, C, H, W = 16, 64, 112, 112
O = 128
BPC = B // N_CORES          # images per core
HP = H + 2                  # padded rows per image plane
WP = W + 2                  # padded cols
NTAPS = 9
RPB = 4                     # output rows per block (free dim = 4*112 = 448)
NBLOCKS = H // RPB          # 28
BAND = 16                   # output rows per output band
NBANDS = H // BAND          # 7

F32 = mybir.dt.float32
F16 = mybir.dt.float16

# input bands over padded rows: (first padded row, nrows). The head band
# covers exactly block 0 so the first matmul can start early; later bands
# are completion-chained serially (the PE consumes rows far slower than
# the serial chain delivers them, and the chain keeps the head window's
# HBM bandwidth for the weight + band-0 transfers).
_IN_BANDS = [(0, 6), (6, 12), (18, 16), (34, 16), (50, 16), (66, 16),
             (82, 16), (98, 16)]


def _conv_body(tc, out_ap, xp_ap, w_ap):
    nc = tc.nc
    from contextlib import ExitStack

    with ExitStack() as ctx:
        xpool = ctx.enter_context(tc.tile_pool(name="xb", bufs=1))
        wpool = ctx.enter_context(tc.tile_pool(name="wt", bufs=1))
        pspool = ctx.enter_context(tc.tile_pool(name="ps", bufs=3, space="PSUM"))
        pdpool = ctx.enter_context(tc.tile_pool(name="pd", bufs=1, space="PSUM"))
        opool = ctx.enter_context(tc.tile_pool(name="ob", bufs=4))
        scpool = ctx.enter_context(tc.tile_pool(name="sc", bufs=1))

        # x planes: partitions [64*im, 64*im+64) hold image im, padded.
        xb = xpool.tile([128, HP, WP], F16)
        # weights: wt[p, t, m] = w[m, p % 64, t] (taps replicated per half)
        wt = wpool.tile([128, NTAPS, O], F16)

        # PE warmup: the HAM clock gate only opens after a full 4096-cycle
        # window (3.4us @ 1.2GHz, free-running phase) of uninterrupted PE
        # activity; cold matmuls run at half clock. Dummy matmuls over
        # memset scratch bridge the gap from body entry (~7.7us) until the
        # first real matmul's data is ready (~10.9us): the PE must stay
        # busy THROUGH that point (an idle gap restarts the window), so
        # slightly overshooting and delaying the real stream by <=107ns
        # per surplus dummy beats undershooting, which costs the whole
        # ~1.9us cold-phase penalty.
        # The dummies must be ONE accumulation group: isolated start/stop
        # matmuls pay a ~128-cycle drain between groups, and that ~50% PE
        # duty cycle never trips the HAM busy detector (observed: 30
        # isolated dummies left the gate closed). Memsets go on GpSimd,
        # which is idle and enters the body earliest.
        sw = scpool.tile([128, O], F16)
        sx = scpool.tile([128, O], F16)
        pdum = pdpool.tile([128, RPB * W], F32)
        nc.gpsimd.memset(sw[:], 0.0)
        nc.gpsimd.memset(sx[:], 0.0)
        # Full 128-row dummies: 64-row (half-array) dummy streams were
        # observed NOT to trip the HAM busy detector at all.
        NDUM = 34
        for i in range(NDUM):
            nc.tensor.matmul(pdum[:, 0:O], sw[:], sx[:],
                             start=(i == 0), stop=(i == NDUM - 1),
                             tile_position=(0, 0))

        # Weight split: taps 0-2 go first on the Sync HWDGE ahead of band 0
        # (same queue, FIFO) so block 0 is gated only by ~270KB; taps 3-8 go
        # on the Scalar HWDGE in parallel and land before block 0 needs them.
        nc.sync.dma_start(out=wt[:, 0:3, :], in_=w_ap[:, 0:3, :])
        nc.scalar.dma_start(out=wt[:, 3:NTAPS, :], in_=w_ap[:, 3:NTAPS, :])

        band_dmas = []
        for bi, (r0, n) in enumerate(_IN_BANDS):
            d = nc.sync.dma_start(
                out=xb[:, r0:r0 + n, :],
                in_=xp_ap[:, r0:r0 + n, :],
            )
            if bi >= 2:
                add_dep_helper(d.ins, band_dmas[bi - 1].ins, reason="band chain")
            band_dmas.append(d)

        ob_tiles = {}
        for p in range(NBLOCKS):
            r = RPB * p
            band = r // BAND
            boff = r - band * BAND
            if boff == 0:
                for im in range(BPC):
                    ob_tiles[im] = opool.tile(
                        [128, BAND, W], F16, name=f"ob{im}_{band}", tag=f"ob{im}"
                    )
            ps = [
                pspool.tile([128, RPB, W], F32, tag=f"ps{im}", name=f"ps{im}_{p}")
                for im in range(BPC)
            ]
            for t in range(NTAPS):
                i, j = divmod(t, 3)
                first, last = t == 0, t == NTAPS - 1
                for im in range(BPC):
                    p0 = 64 * im
                    nc.tensor.matmul(
                        ps[im][:],
                        wt[p0:p0 + 64, t, :],
                        xb[p0:p0 + 64, r + i:r + i + RPB, j:j + W],
                        start=first,
                        stop=last,
                        tile_position=(p0, 0),
                    )
            # PSUM -> SBUF casts split across engines: image 0 on DVE,
            # image 1 on ACT (both ~1 elem/cycle from PSUM), so the two
            # copies run concurrently and the end-of-kernel drain chain
            # is one copy deep, not two.
            nc.vector.tensor_copy(ob_tiles[0][:, boff:boff + RPB, :], ps[0][:])
            nc.scalar.copy(ob_tiles[1][:, boff:boff + RPB, :], ps[1][:])
            last_band = band == NBANDS - 1
            if last_band:
                # store per block; image 0 on the Sync queue (input bands
                # have long drained), image 1 on Scalar -> the two final
                # stores run on independent HWDGE rings.
                nc.sync.dma_start(
                    out=out_ap[0, :, r:r + RPB, :],
                    in_=ob_tiles[0][:, boff:boff + RPB, :],
                )
                nc.scalar.dma_start(
                    out=out_ap[1, :, r:r + RPB, :],
                    in_=ob_tiles[1][:, boff:boff + RPB, :],
                )
            elif boff + RPB == BAND:
                for im in range(BPC):
                    nc.scalar.dma_start(
                        out=out_ap[im, :, band * BAND:(band + 1) * BAND, :],
                        in_=ob_tiles[im][:],
                    )


def build_program():
    nc = bacc.Bacc("TRN2", target_bir_lowering=False, num_devices=N_CORES)
    x_t = nc.dram_tensor("xp", [BPC * C, HP, WP], F16, kind="ExternalInput")
    w_t = nc.dram_tensor("wT", [128, NTAPS, O], F16, kind="ExternalInput")
    o_t = nc.dram_tensor("out", [BPC, O, H, W], F16, kind="ExternalOutput")
    with tile.TileContext(nc) as tc:
        _conv_body(tc, o_t.ap(), x_t.ap(), w_t.ap())
    nc.compile()
    return nc


def pack_weights(weights: np.ndarray) -> np.ndarray:
    # (O, C, 9) -> (128, 9, O) with wT[p, t, m] = weights[m, p % 64, t]
    wT = np.ascontiguousarray(np.transpose(weights, (1, 2, 0)))  # (C, 9, O)
    return np.ascontiguousarray(
        np.concatenate([wT, wT], axis=0).astype(np.float16)
    )


def pad_input(x: np.ndarray) -> np.ndarray:
    # (B, C, H, W) -> (B, C, H+2, W+2) zero-padded, fp16
    xp = np.zeros((x.shape[0], x.shape[1], HP, WP), np.float16)
    xp[:, :, 1:1 + H, 1:1 + W] = x
    return xp


def run(x: np.ndarray, weights: np.ndarray, **spmd_kwargs):
    x = np.ascontiguousarray(x, dtype=np.float32)
    w = np.ascontiguousarray(weights, dtype=np.float32)
    wT = pack_weights(w)
    xp = pad_input(x)
    nc = build_program()
    in_maps = [
        {"xp": xp[BPC * i:BPC * (i + 1)].reshape(BPC * C, HP, WP), "wT": wT}
        for i in range(N_CORES)
    ]
    res = run_bass_kernel_spmd(nc, in_maps, list(range(N_CORES)), **spmd_kwargs)
    outs = [
        np.asarray(res.results[i]["out"])
        .reshape(BPC, O, H, W)
        .astype(np.float32)
        for i in range(N_CORES)
    ]
    return np.concatenate(outs, axis=0), res


def kernel(x: np.ndarray, weights: np.ndarray) -> np.ndarray:
    out, _ = run(x, weights)
    return out
